# revision 13
# baseline (speedup 1.0000x reference)
"""Trainium2 Bass kernel for nn_BaselineAttention_36172214567310 (v4).

Reference computation (einsum 'bhqk,bhkd->bhkd' sums over q, so attention
collapses to: v scaled by softmax column-sums):

    qkv = x @ w_qkv
    P = softmax(q @ k^T / 8)      per head, rows sum to 1
    colsum[k] = sum_q P[q, k]
    out = (v * colsum[:, None]) @ w_o

Sharding: 8 cores = 2 batches x 4 head-groups (4 heads each).

v7 = v6 - LDW padding (measured: hurts) + bf16 outputs (the three
partial outputs totalled 24MB fp32 of DMA writes and made the tail
DMA-bound; bf16 halves it, host sums in fp32).
v5 = v4 + q/k projections in fp8 DoubleRow (half the MMs, shorter lead).
v4: uniform PE instruction density to keep the HAM clock-gate warm:
- warmup MMs at t=0 (no DMA dependency) so the lead projection runs at
  2.4 GHz; lead is only q01-h0 + k01.
- one global filler queue (rest of the qkv projection, then the output
  projection in four readiness-gated quarter phases) paced at ~3.6
  instructions per chunk under the scores+exp+matvec steady loop.
- output projection is split into three DRAM outputs (v01 x wo0 k-half,
  v23-head2 rows, v23-head3 rows) summed on the host, so nearly all of
  P4 streams out mid-kernel.
- lagged DoubleRow fp8 matvec (no bursts): head j k-half1 runs during
  head j+1 chunks 0-7, k-half0 during own chunks 8-15.
"""

import sys

sys.path.insert(0, "/opt/trn_rl_repo")

import numpy as np

B, S, HIDDEN = 2, 2048, 1024
NH, HD = 16, 64
HPC = 4
N_CORES = 8
P = 128
QC = S // P
NPAIR = QC // 2

_CACHE = {}


def _build():
    if "nc" in _CACHE:
        return _CACHE["nc"]

    import concourse.mybir as mybir
    import concourse.tile as tile
    from concourse import bacc

    F32 = mybir.dt.float32
    BF16 = mybir.dt.bfloat16
    FP8 = mybir.dt.float8e4
    EXP = mybir.ActivationFunctionType.Exp
    COPY = mybir.ActivationFunctionType.Copy
    ADD = mybir.AluOpType.add
    MULT = mybir.AluOpType.mult
    DR = mybir.MatmulPerfMode.DoubleRow

    nc = bacc.Bacc()
    xT_d = nc.declare_dram_parameter("xT", [HIDDEN, S], BF16, isOutput=False)
    x8_d = nc.declare_dram_parameter("x8", [HIDDEN // 2, 2 * S], FP8, isOutput=False)
    w8_d = nc.declare_dram_parameter("w8", [HIDDEN // 2, 2 * 512], FP8, isOutput=False)
    wqkv_d = nc.declare_dram_parameter("wqkv", [HIDDEN, 256], BF16, isOutput=False)
    wo_d = nc.declare_dram_parameter("wo", [256, HIDDEN], BF16, isOutput=False)
    out_d = nc.declare_dram_parameter("out", [S, HIDDEN], BF16, isOutput=True)
    out2_d = nc.declare_dram_parameter("out2", [S, HIDDEN], BF16, isOutput=True)
    out3_d = nc.declare_dram_parameter("out3", [S, HIDDEN], BF16, isOutput=True)

    with tile.TileContext(nc) as tc:
        with tc.tile_pool(name="persist", bufs=1) as sb, \
             tc.tile_pool(name="small", bufs=1) as sm, \
             tc.tile_pool(name="rsp", bufs=8) as rsp, \
             tc.tile_pool(name="outp", bufs=3) as outp, \
             tc.tile_pool(name="ps_s", bufs=2, space="PSUM") as ps_s_pool, \
             tc.tile_pool(name="ps_c", bufs=1, space="PSUM") as ps_c_pool, \
             tc.tile_pool(name="ps_f", bufs=1, space="PSUM") as ps_f_pool:

            xt = [sb.tile([P, S], BF16, name=f"xt{kc}") for kc in range(8)]
            x8t = [sb.tile([P, 2, S], FP8, name=f"x8t{p}") for p in range(4)]
            w8t = [sb.tile([P, 2, 512], FP8, name=f"w8t{p}") for p in range(4)]
            wq_t = [sb.tile([P, 256], BF16, name=f"wq{kc}") for kc in range(8)]
            wo_t = [sb.tile([P, HIDDEN], BF16, name=f"wo{kc}") for kc in range(2)]
            qkvt = [sb.tile([P, S], BF16, name=f"qkvt{mc}") for mc in range(6)]
            e8 = [sb.tile([P, QC, S], FP8, name=f"e8_{i}") for i in range(2)]
            wr8 = [sb.tile([P, QC, P], FP8, name=f"wr8_{i}") for i in range(2)]
            dum = sm.tile([P, 1], F32, name="dum")
            dum2 = sm.tile([P, 1], F32, name="dum2")
            wsrc = sm.tile([P, 512], BF16, name="wsrc")

            # exp table preload + PE warmup (no DMA dependency)
            nc.vector.memset(dum, 0.0)
            nc.scalar.activation(dum2, dum, EXP)
            nc.vector.memset(wsrc, 0.0)
            wps = ps_c_pool.tile([P, 1024], F32, name="psc")
            for i in range(20):
                nc.tensor.matmul(wps[:, 0:512], wsrc[:, 0:128], wsrc,
                                 start=True, stop=True)

            for p in range(4):
                nc.sync.dma_start(out=w8t[p], in_=w8_d[p * P:(p + 1) * P, :])
                nc.sync.dma_start(out=x8t[p], in_=x8_d[p * P:(p + 1) * P, :])
            for kc in range(8):
                nc.sync.dma_start(out=wq_t[kc],
                                  in_=wqkv_d[kc * P:(kc + 1) * P, :])
                nc.sync.dma_start(out=xt[kc], in_=xT_d[kc * P:(kc + 1) * P, :])
            for kc in range(2):
                nc.sync.dma_start(out=wo_t[kc],
                                  in_=wo_d[kc * P:(kc + 1) * P, :])

            # ---------------- global filler queue ----------------
            # p1 item: ("p1", ready, mc, hh, kc, n) - one 512-col MM of the
            #   qkv projection (16 MMs per (mc,hh) accumulation + copy).
            # p4 item: ("p4", ready, which, sc, n) - output projection MM.
            queue = []
            for dk in range(4):                      # q01-h1 (DR), deadline gc 8
                for n in range(2):
                    queue.append(("qk", 0, 0, 1, dk, n))
            for mc in (2,):                          # v01 (bf16)
                for hh in range(2):
                    for kc in range(8):
                        for n in range(2):
                            queue.append(("p1", 0, mc, hh, kc, n))
            for mc in (1, 2):                        # q23, k23 (DR; m-block 2,3... mapped below)
                for hh in range(2):
                    for dk in range(4):
                        for n in range(2):
                            queue.append(("qk", 0, mc + 1, hh, dk, n))
            for mc in (5,):                          # v23 (bf16)
                for hh in range(2):
                    for kc in range(8):
                        for n in range(2):
                            queue.append(("p1", 0, mc, hh, kc, n))
            # p4 quarter phases:
            # A: out2 = v01^T x wo0            (full K=128), sc 0-7 ready 33,
            #    sc 8-15 ready 41
            # B: out3 = v23[head2 rows] x wo1  (K=64),      sc 0-7 ready 49,
            #    sc 8-15 ready 57
            for sc in range(8):
                for n in range(2):
                    queue.append(("p4", 33, "A", sc, n))
            for sc in range(8, QC):
                for n in range(2):
                    queue.append(("p4", 41, "A", sc, n))
            for sc in range(8):
                for n in range(2):
                    queue.append(("p4", 49, "B", sc, n))
            for sc in range(8, QC):
                for n in range(2):
                    queue.append(("p4", 57, "B", sc, n))

            f_state = {"i": 0, "ps": None, "ps4": None}

            def emit_item(it):
                if it[0] == "qk":
                    _, _, mb, hh, dk, n = it
                    if dk == 0 and n == 0:
                        f_state["ps"] = ps_f_pool.tile([P, 1024], F32,
                                                       name="psf")
                    ps = f_state["ps"]
                    c0 = hh * 1024 + n * 512
                    nc.tensor.matmul(
                        ps[:, n * 512:(n + 1) * 512],
                        w8t[dk][:, :, mb * P:(mb + 1) * P],
                        x8t[dk][:, :, c0:c0 + 512],
                        perf_mode=DR,
                        start=(dk == 0), stop=(dk == 3))
                    if dk == 3 and n == 1:
                        qdst = {0: 0, 2: 3, 3: 4}[mb]
                        nc.vector.tensor_copy(
                            out=qkvt[qdst][:, hh * 1024:(hh + 1) * 1024],
                            in_=ps)
                    return
                if it[0] == "p1":
                    _, _, mc, hh, kc, n = it
                    if kc == 0 and n == 0:
                        f_state["ps"] = ps_f_pool.tile([P, 1024], F32,
                                                       name="psf")
                    ps = f_state["ps"]
                    c0 = hh * 1024 + n * 512
                    wcol = 0 if mc == 2 else P
                    nc.tensor.matmul(
                        ps[:, n * 512:(n + 1) * 512],
                        wq_t[kc][:, wcol:wcol + P],
                        xt[kc][:, c0:c0 + 512],
                        start=(kc == 0), stop=(kc == 7))
                    if kc == 7 and n == 1:
                        nc.vector.tensor_copy(
                            out=qkvt[mc][:, hh * 1024:(hh + 1) * 1024], in_=ps)
                else:
                    _, _, which, sc, n = it
                    if n == 0:
                        f_state["ps4"] = ps_f_pool.tile([P, 1024], F32,
                                                        name="psf")
                    ps4 = f_state["ps4"]
                    if which == "A":
                        lhsT = qkvt[2][:, sc * P:(sc + 1) * P]
                        rhs = wo_t[0][:, n * 512:(n + 1) * 512]
                        dst = out2_d
                    else:
                        lhsT = qkvt[5][0:64, sc * P:(sc + 1) * P]
                        rhs = wo_t[1][0:64, n * 512:(n + 1) * 512]
                        dst = out3_d
                    nc.tensor.matmul(ps4[:, n * 512:(n + 1) * 512], lhsT, rhs,
                                     start=True, stop=True)
                    if n == 1:
                        o_sb = outp.tile([P, HIDDEN], BF16, name="osb")
                        nc.vector.tensor_copy(out=o_sb, in_=ps4)
                        nc.sync.dma_start(out=dst[sc * P:(sc + 1) * P, :],
                                          in_=o_sb)

            def pump(gc, budget):
                while budget > 0 and f_state["i"] < len(queue):
                    it = queue[f_state["i"]]
                    if it[1] > gc:
                        return
                    f_state["i"] += 1
                    emit_item(it)
                    budget -= 1

            def target(gc):
                if gc < 8:
                    return int(4.5 * (gc + 1))
                if gc <= 46:
                    return 36 + int(2.75 * (gc - 7))
                return min(len(queue), 137 + 2 * (gc - 46))

            # ---------------- lead: q01-h0, k01 ----------------
            def emit_lead(mb, qdst, hh, on_act):
                ps = ps_s_pool.tile([P, 1024], F32, name="pss")
                for dk in range(4):
                    for n in range(2):
                        c0 = hh * 1024 + n * 512
                        nc.tensor.matmul(
                            ps[:, n * 512:(n + 1) * 512],
                            w8t[dk][:, :, mb * P:(mb + 1) * P],
                            x8t[dk][:, :, c0:c0 + 512],
                            perf_mode=DR,
                            start=(dk == 0), stop=(dk == 3))
                dst = qkvt[qdst][:, hh * 1024:(hh + 1) * 1024]
                if on_act:
                    nc.scalar.activation(dst, ps, COPY)
                else:
                    nc.vector.tensor_copy(out=dst, in_=ps)

            emit_lead(0, 0, 0, True)
            emit_lead(1, 1, 0, False)
            emit_lead(1, 1, 1, False)

            # ---------------- head loop ----------------
            def matvec(j, half, pr, first, last):
                eb, wb = e8[j % 2], wr8[j % 2]
                psc = f_state["psc"]
                for n in range(2):
                    c0 = half * 1024 + n * 512
                    nc.tensor.matmul(
                        psc[:, n * 512:(n + 1) * 512],
                        wb[:, 2 * pr:2 * pr + 2, :],
                        eb[:, 2 * pr:2 * pr + 2, c0:c0 + 512],
                        perf_mode=DR,
                        start=first, stop=last)

            def vscale(j, half):
                vt = qkvt[2 if j < 2 else 5]
                bp = (j % 2) * 64
                psc = f_state["psc"]
                c0 = half * 1024
                nc.vector.tensor_tensor(
                    vt[bp:bp + 64, c0:c0 + 1024], vt[bp:bp + 64, c0:c0 + 1024],
                    psc[bp:bp + 64, :], MULT)

            for j in range(HPC):
                qt = qkvt[0 if j < 2 else 3]
                kt = qkvt[1 if j < 2 else 4]
                bp = (j % 2) * 64
                eb, wb = e8[j % 2], wr8[j % 2]

                for qc in range(QC):
                    gc = j * QC + qc
                    # DVE-light slots: rowsum via one fp8 tensor_reduce on
                    # DVE instead of two ACT accumulator reads (~574ns off
                    # the pacing engine per slot)
                    dve_rs = j < 2 and 2 <= qc <= 9
                    r_h = [None, None]
                    for hh in range(2):
                        ps_s = ps_s_pool.tile([P, 1024], F32, name="pss")
                        for n in range(2):
                            c0 = hh * 1024 + n * 512
                            nc.tensor.matmul(
                                ps_s[:, n * 512:(n + 1) * 512],
                                qt[bp:bp + 64, qc * P:(qc + 1) * P],
                                kt[bp:bp + 64, c0:c0 + 512],
                                start=True, stop=True)
                        if dve_rs:
                            nc.scalar.activation(
                                eb[:, qc, hh * 1024:(hh + 1) * 1024],
                                ps_s, EXP, scale=0.125)
                        else:
                            r = rsp.tile([P, 1], F32, name=f"r{hh}")
                            nc.scalar.activation(
                                eb[:, qc, hh * 1024:(hh + 1) * 1024],
                                ps_s, EXP, scale=0.125, accum_out=r)
                            r_h[hh] = r
                        if hh == 0:
                            due = target(gc) - f_state["i"]
                            pump(gc, max(0, min(3, (due + 1) // 2)))
                    rs = rsp.tile([P, 1], F32, name="rs")
                    if dve_rs:
                        nc.vector.tensor_reduce(
                            rs, eb[:, qc, :], mybir.AxisListType.XY,
                            ADD)
                    else:
                        nc.vector.tensor_tensor(rs, r_h[0], r_h[1], ADD)
                    rinv = rsp.tile([P, 1], F32, name="rinv")
                    nc.vector.reciprocal(rinv, rs)
                    nc.vector.tensor_scalar(wb[:, qc, :],
                                            rinv.to_broadcast([P, P]),
                                            1024.0, None, MULT)
                    has_mv = (qc < NPAIR and j > 0) or qc >= NPAIR
                    if qc < NPAIR and j > 0:
                        if qc == 0:
                            f_state["psc"] = ps_c_pool.tile(
                                [P, 1024], F32, name="psc")
                        matvec(j - 1, 1, qc, qc == 0, qc == NPAIR - 1)
                        if qc == NPAIR - 1:
                            vscale(j - 1, 1)
                    elif qc >= NPAIR:
                        pr = qc - NPAIR
                        if pr == 0:
                            f_state["psc"] = ps_c_pool.tile(
                                [P, 1024], F32, name="psc")
                        matvec(j, 0, pr, pr == 0, pr == NPAIR - 1)
                        if pr == NPAIR - 1:
                            vscale(j, 0)
                    pump(gc, max(0, min(5, target(gc) - f_state["i"])))

            # ---------------- tail ----------------
            # head 3 k-half1 matvec interleaved with out += v23[head3] x wo1
            # for s-chunks 0-7 (those only need the k-half0 v-scale, done).
            def p4bb(sc):
                ps_o = ps_s_pool.tile([P, 1024], F32, name="pss")
                for n in range(2):
                    nc.tensor.matmul(
                        ps_o[:, n * 512:(n + 1) * 512],
                        qkvt[5][64:128, sc * P:(sc + 1) * P],
                        wo_t[1][64:128, n * 512:(n + 1) * 512],
                        start=True, stop=True)
                o_sb = outp.tile([P, HIDDEN], BF16, name="osb")
                if sc % 2 == 0:
                    nc.scalar.activation(o_sb, ps_o, COPY)
                else:
                    nc.vector.tensor_copy(out=o_sb, in_=ps_o)
                nc.sync.dma_start(out=out_d[sc * P:(sc + 1) * P, :], in_=o_sb)

            f_state["psc"] = ps_c_pool.tile([P, 1024], F32, name="psc")
            for pr in range(NPAIR):
                matvec(3, 1, pr, pr == 0, pr == NPAIR - 1)
                pump(63, 2)
                p4bb(pr)
            vscale(3, 1)
            pump(63, len(queue))
            for sc in range(NPAIR, QC):
                p4bb(sc)

    nc.compile()
    _CACHE["nc"] = nc
    return nc


def kernel(x: np.ndarray, w_qkv: np.ndarray, w_o: np.ndarray) -> np.ndarray:
    import ml_dtypes
    from concourse.bass_utils import run_bass_kernel_spmd

    nc = _build()

    def pair_interleave(a):
        # [1024, C] -> [512, 2C]: rows 256p+128i+part -> row 128p+part,
        # col block i
        cc = a.shape[1]
        return np.ascontiguousarray(
            a.reshape(4, 2, 128, cc).transpose(0, 2, 1, 3).reshape(512, 2 * cc))

    def to_fp8(a):
        return np.clip(a, -240.0, 240.0).astype(ml_dtypes.float8_e4m3)

    xT = [np.ascontiguousarray(x[b].T).astype(ml_dtypes.bfloat16)
          for b in range(B)]
    x8 = [to_fp8(pair_interleave(np.ascontiguousarray(x[b].T)))
          for b in range(B)]
    in_maps = []
    for c in range(N_CORES):
        b, g = divmod(c, HPC)
        base = 256 * g
        q01 = w_qkv[:, base:base + 128]
        q23 = w_qkv[:, base + 128:base + 256]
        k01 = w_qkv[:, 1024 + base:1024 + base + 128]
        k23 = w_qkv[:, 1024 + base + 128:1024 + base + 256]
        v01 = w_qkv[:, 2048 + base:2048 + base + 128]
        v23 = w_qkv[:, 2048 + base + 128:2048 + base + 256]
        wqk = np.concatenate([q01, k01, q23, k23], axis=1)
        wv = np.concatenate([v01, v23], axis=1)
        wo_slice = w_o[base:base + 256, :] * (1.0 / 1024.0)
        in_maps.append({
            "xT": xT[b],
            "x8": x8[b],
            "w8": to_fp8(pair_interleave(wqk)),
            "wqkv": wv.astype(ml_dtypes.bfloat16),
            "wo": wo_slice.astype(ml_dtypes.bfloat16),
        })

    res = run_bass_kernel_spmd(nc, in_maps, list(range(N_CORES)),
                               **_CACHE.get("run_kwargs", {}))
    _CACHE["last_result"] = res

    out = np.zeros((B, S, HIDDEN), np.float32)
    for c in range(N_CORES):
        r = res.results[c]
        out[c // HPC] += (r["out"].astype(np.float32)
                          + r["out2"].astype(np.float32)
                          + r["out3"].astype(np.float32))
    return out

